# revision 25
# baseline (speedup 1.0000x reference)
"""Trainium2 Bass kernel for nn_NodeEncodeInterface (GNN message passing).

Strategy (per sharding hint: shard nodes/edges with graph-partitioned edge
cuts, replicate small embeddings + MLP weights):
 - Host: partitions valid carbon->hydrogen edges by owner core (src chunk),
   packs them into static 128-edge columns (<=RPC carbon ranks per column),
   and ships ONLY the x rows each core actually touches, already laid out in
   the packed edge/carbon slot order (fp16 wire format).  The solvent
   embedding is pre-concatenated into each 288-dim feature row, and 1/deg is
   folded into the edge weight, so the device needs no gather, no transpose,
   and no divide.  MLP weights ride inside the NEFF as Const tensors
   (loaded at model-load time, not per-execute).
 - Device (8 NeuronCores, SPMD): computes the segment-mean via
   selection-matrix matmuls in PSUM (fp16 operands, fp32 accumulate), then
   runs both Projection MLPs in transposed orientation, emitting compact
   per-carbon outputs.
 - Host: scatters compact outputs into the full [N, 2] result.
"""

import hashlib

import numpy as np

import concourse.bass as bass
import concourse.mybir as mybir
import concourse.tile as tile_mod
from concourse.tile import TileContext
from concourse.vector_clock import ScopedClock
from concourse import bass_utils

f32 = mybir.dt.float32
f16 = mybir.dt.float16
ALU = mybir.AluOpType

N = 300000
HID = 256
EMB = 32
FH = EMB + HID            # 288 feature dim (emb ++ x)
C2 = FH - 256             # 32: last lhsT chunk of the 288-dim contraction
NCORES = 8
CH = N // NCORES          # 37500 nodes per core

# geometry ladder: smallest (ranks-per-column, n-columns) that fits the
# per-core packing is chosen at runtime (deterministic inputs -> first entry)
GEOMS = ((112, 15), (128, 16), (128, 24), (128, 48))


def _geom_params(rpc, ncol):
    slots = rpc * ncol
    ngrp = -(-slots // 512)
    while slots % ngrp:
        ngrp += 1
    return slots, slots // ngrp, ngrp


# ---------------------------------------------------------------------------
# walrus workaround: this build rejects >1 semaphore wait on several lowered
# instruction encodings; split extra waits onto same-engine NoOps.
# ---------------------------------------------------------------------------
def _patched_drain_and_barrier(self, tick_clock, wait_clock):
    nc = self.nc
    drain_inst = nc.sync.drain()
    wait_clock.add_sem_waits(
        drain_inst.ins, ScopedClock({None: tick_clock.global_clock})
    )
    si = drain_inst.ins.sync_info
    waits = list(si.on_wait)
    if len(waits) > 1:
        si.on_wait = waits[:1]
        for w in waits[1:]:
            extra = nc.sync.drain()
            extra.ins.sync_info = mybir.SyncInfo(on_wait=[w], on_update=[])
    nc.all_engine_barrier()
    popped = nc._tile_sem_poison_stack.pop()
    assert popped is self._sem_poison
    nc.clear_and_free_semaphores(list(self.sems.allocated().values()))
    nc.all_engine_barrier()


tile_mod.TileContext._drain_and_barrier = _patched_drain_and_barrier


def _split_waits(nc, maxw=1):
    fn = nc.m.functions[0]
    for bb in fn.blocks:
        out = []
        changed = False
        for inst in bb.instructions:
            si = inst.sync_info
            waits = list(si.on_wait) if si is not None else []
            if len(waits) > maxw:
                changed = True
                for i in range(0, len(waits) - maxw, maxw):
                    nop = mybir.InstNoOp(
                        name=nc.get_next_instruction_name(),
                        text_hint="waitsplit",
                        bass_nofuse=True,
                    )
                    nop.engine = inst.engine
                    nop.sync_info = mybir.SyncInfo(
                        on_wait=waits[i : i + maxw], on_update=[]
                    )
                    out.append(nop)
                si.on_wait = waits[len(waits) - maxw :]
            out.append(inst)
        if changed:
            bb.instructions[:] = out
    return nc


# ---------------------------------------------------------------------------
# device kernel
# ---------------------------------------------------------------------------
NSOLV = 9


def _build(rpc, ncol, wdata):
    SLOTS, GRP, NGRP = _geom_params(rpc, ncol)
    nc = bass.Bass("TRN2")
    # per-core packed node features (host prepared, fp16); features are the
    # 256-dim x rows only -- the 32-dim solvent embedding is applied via
    # one-hot matmuls against host-precomputed (emb @ W1[:32]) const tables
    xg = nc.dram_tensor("xg", [128, ncol * HID], f16, kind="ExternalInput")
    xc01 = nc.dram_tensor("xc01", [128, 2 * SLOTS], f16, kind="ExternalInput")
    oh9c = nc.dram_tensor("oh9c", [NSOLV, SLOTS], f16, kind="ExternalInput")
    vlvw = nc.dram_tensor("vlvw", [128, 3 * ncol], f32, kind="ExternalInput")
    # replicated MLP weights: Const tensors embedded in the NEFF
    wmain_d = nc.inline_tensor(wdata["wmain"], name="k_wmain")
    ue9_d = nc.inline_tensor(wdata["ue9"], name="k_ue9")
    bias_d = nc.inline_tensor(wdata["bias"], name="k_bias")
    out = nc.dram_tensor("out", [2, SLOTS], f32, kind="ExternalOutput")

    with TileContext(nc) as tc:
        with (
            tc.tile_pool(name="const", bufs=1) as cst,
            tc.tile_pool(name="wts", bufs=1) as wts,
            tc.tile_pool(name="edge", bufs=1) as edg,
            tc.tile_pool(name="work", bufs=3) as wrk,
            tc.tile_pool(name="hsum", bufs=1) as hsp,
            tc.tile_pool(name="mlp", bufs=2) as mlp,
            tc.tile_pool(name="psE", bufs=2, space="PSUM") as psE,
            tc.tile_pool(name="psS", bufs=1, space="PSUM") as psS,
            tc.tile_pool(name="psL", bufs=2, space="PSUM") as psL,
            tc.tile_pool(name="outp", bufs=1) as outp,
        ):
            iota = cst.tile([128, rpc], mybir.dt.int32)
            nc.gpsimd.iota(iota[:], pattern=[[1, rpc]], base=0, channel_multiplier=0)
            iotaf = cst.tile([128, rpc], f32)
            nc.vector.tensor_copy(iotaf[:], iota[:])
            iota9 = cst.tile([128, NSOLV], mybir.dt.int32)
            nc.gpsimd.iota(iota9[:], pattern=[[1, NSOLV]], base=0, channel_multiplier=0)
            iota9f = cst.tile([128, NSOLV], f32)
            nc.vector.tensor_copy(iota9f[:], iota9[:])

            # ---- inputs to SBUF, ordered so the c-side MLP (which needs
            # only w1/xc01) can start while xg/w2 still stream in ----
            wm = wts.tile([128, 2 * 1540], f16)
            nc.sync.dma_start(out=wm[:, 0:1024], in_=wmain_d[:, 0:1024])
            ue9 = wts.tile([NSOLV, 512], f16)
            nc.sync.dma_start(out=ue9[:], in_=ue9_d[:])
            bias = wts.tile([128, 16], f32)
            nc.sync.dma_start(out=bias[:], in_=bias_d[:])
            vlvwT = edg.tile([128, 3 * ncol], f32)
            nc.sync.dma_start(out=vlvwT[:], in_=vlvw[:])
            oh9cT = edg.tile([NSOLV, SLOTS], f16)
            nc.sync.dma_start(out=oh9cT[:], in_=oh9c[:])
            # xc01 and the w2 block stream in halves so group-0 compute can
            # start before the full transfer lands
            xc01T = edg.tile([128, 2 * SLOTS], f16)
            half = 2 * GRP
            nc.sync.dma_start(
                out=xc01T[:].rearrange("p (c s) -> p c s", c=2)[:, :, 0:half],
                in_=xc01[:].rearrange("p (c s) -> p c s", c=2)[:, :, 0:half],
            )
            nc.sync.dma_start(out=wm[:, 1024:2048], in_=wmain_d[:, 1024:2048])
            nc.sync.dma_start(
                out=xc01T[:].rearrange("p (c s) -> p c s", c=2)[:, :, half:SLOTS],
                in_=xc01[:].rearrange("p (c s) -> p c s", c=2)[:, :, half:SLOTS],
            )
            nc.sync.dma_start(out=wm[:, 2048:3080], in_=wmain_d[:, 2048:3080])
            xgT = edg.tile([128, ncol * HID], f16)
            nc.sync.dma_start(out=xgT[:], in_=xg[:])

            hs01T = hsp.tile([128, 2 * SLOTS], f16)
            hs9T = hsp.tile([NSOLV, SLOTS], f16)
            o2c = outp.tile([1, SLOTS], f32)
            o2h = outp.tile([1, SLOTS], f32)

            h1t = {}
            h2t = {}
            p3t = {}

            def emit_l1(s, wo, r01, r9, bo, g):
                gs = slice(g * GRP, (g + 1) * GRP)
                gs1 = slice(SLOTS + g * GRP, SLOTS + (g + 1) * GRP)
                h1t[s, g] = mlp.tile([128, 2 * GRP], f16, tag="h1" + s, name="h1t" + s)
                b1o = 512 * wo
                for fb in range(2):
                    ph = psL.tile([128, GRP], f32, tag="pl1")
                    nc.tensor.matmul(ph[:], lhsT=wm[:, b1o + fb * 128 : b1o + fb * 128 + 128], rhs=r01[:, gs], start=True, stop=False)
                    nc.tensor.matmul(ph[:], lhsT=wm[:, b1o + 256 + fb * 128 : b1o + 256 + fb * 128 + 128], rhs=r01[:, gs1], start=False, stop=False)
                    nc.tensor.matmul(ph[:], lhsT=ue9[:, 256 * wo + fb * 128 : 256 * wo + (fb + 1) * 128], rhs=r9[:, gs], start=False, stop=True)
                    nc.vector.tensor_scalar(
                        out=h1t[s, g][:, fb * GRP : (fb + 1) * GRP], in0=ph[:],
                        scalar1=bias[:, bo + fb : bo + fb + 1], scalar2=None, op0=ALU.add,
                    )

            def emit_l2l3(s, wo, o2, bo, g):
                gs = slice(g * GRP, (g + 1) * GRP)
                h2t[s, g] = mlp.tile([128, 4 * GRP], f16, tag="h2" + s, name="h2t" + s)
                p3t[s, g] = psS.tile([1, GRP], f32, tag="p3" + s, name="p3t" + s)
                b2o = 1024 + 1024 * wo
                for fb in range(4):
                    p2m = psL.tile([128, GRP], f32, tag="pl2")
                    nc.tensor.matmul(p2m[:], lhsT=wm[:, b2o + fb * 128 : b2o + (fb + 1) * 128], rhs=h1t[s, g][:, 0:GRP], start=True, stop=False)
                    nc.tensor.matmul(p2m[:], lhsT=wm[:, b2o + 512 + fb * 128 : b2o + 512 + (fb + 1) * 128], rhs=h1t[s, g][:, GRP : 2 * GRP], start=False, stop=True)
                    nc.scalar.activation(
                        h2t[s, g][:, fb * GRP : (fb + 1) * GRP], p2m[:],
                        mybir.ActivationFunctionType.Relu, bias=bias[:, bo + 2 + fb : bo + 3 + fb],
                    )
                for fb in range(4):
                    nc.tensor.matmul(
                        p3t[s, g][:], lhsT=wm[:, 3072 + 4 * wo + fb : 3072 + 4 * wo + fb + 1],
                        rhs=h2t[s, g][:, fb * GRP : (fb + 1) * GRP],
                        start=(fb == 0), stop=(fb == 3),
                    )
                nc.vector.tensor_scalar(
                    out=o2[:, gs], in0=p3t[s, g][:],
                    scalar1=bias[0:1, bo + 6 : bo + 7], scalar2=None, op0=ALU.add,
                )

            def emit_side(s, wo, r01, r9, o2, bo):
                for g in range(NGRP):
                    emit_l1(s, wo, r01, r9, bo, g)
                    if g >= 1:
                        emit_l2l3(s, wo, o2, bo, g - 1)
                emit_l2l3(s, wo, o2, bo, NGRP - 1)

            # ---- c-side MLP first: overlaps the xg/w2 input stream ----
            emit_side("c", 0, xc01T, oh9cT, o2c, 0)
            nc.sync.dma_start(out=out[0:1, :], in_=o2c[:])

            # ---- segment mean via selection matmuls ----
            for i in range(ncol):
                Seq = wrk.tile([128, rpc], f32, tag="Seq")
                nc.vector.tensor_tensor(
                    out=Seq[:],
                    in0=vlvwT[:, i : i + 1].to_broadcast([128, rpc]),
                    in1=iotaf[:],
                    op=ALU.is_equal,
                )
                S16 = wrk.tile([128, rpc], f16, tag="S16")
                nc.vector.tensor_scalar(
                    out=S16[:], in0=Seq[:], scalar1=vlvwT[:, ncol + i : ncol + i + 1],
                    scalar2=None, op0=ALU.mult,
                )
                H9 = wrk.tile([128, NSOLV], f16, tag="H9")
                nc.vector.tensor_tensor(
                    out=H9[:],
                    in0=vlvwT[:, 2 * ncol + i : 2 * ncol + i + 1].to_broadcast([128, NSOLV]),
                    in1=iota9f[:],
                    op=ALU.is_equal,
                )
                sl = slice(i * rpc, (i + 1) * rpc)
                pE = psE.tile([128, 3 * rpc], f32, tag="pE")
                base = i * HID
                nc.tensor.matmul(pE[:, 0:rpc], lhsT=xgT[:, base : base + 128], rhs=S16[:], start=True, stop=True)
                nc.tensor.matmul(pE[:, rpc : 2 * rpc], lhsT=xgT[:, base + 128 : base + 256], rhs=S16[:], start=True, stop=True)
                nc.tensor.matmul(pE[0:NSOLV, 2 * rpc : 3 * rpc], lhsT=H9[:], rhs=S16[:], start=True, stop=True)
                nc.vector.tensor_copy(
                    hs01T[:].rearrange("p (c s) -> p c s", c=2)[:, :, sl],
                    pE[:, 0 : 2 * rpc].rearrange("p (c r) -> p c r", c=2),
                )
                nc.vector.tensor_copy(hs9T[:, sl], pE[0:NSOLV, 2 * rpc : 3 * rpc])

            # ---- h-side MLP ----
            emit_side("h", 1, hs01T, hs9T, o2h, 8)
            nc.sync.dma_start(out=out[1:2, :], in_=o2h[:])
    _split_waits(nc)
    return nc


_NC_CACHE = {}


def _get_nc(rpc, ncol, wdata):
    h = hashlib.sha1()
    for k in sorted(wdata):
        h.update(k.encode())
        h.update(wdata[k].tobytes())
    key = (rpc, ncol, h.hexdigest())
    if key not in _NC_CACHE:
        _NC_CACHE[key] = _build(rpc, ncol, wdata)
    return _NC_CACHE[key]


# ---------------------------------------------------------------------------
# host side
# ---------------------------------------------------------------------------
def _pack(counts, rpc):
    """Greedy packing of nodes (with edge multiplicities `counts`) into
    columns of <=128 edges and <=rpc ranks.  Returns (node_col, node_rank)."""
    k = len(counts)
    node_col = np.zeros(k, np.int32)
    node_rank = np.zeros(k, np.int32)
    col = 0
    col_edges = 0
    col_ranks = 0
    for j in range(k):
        d = counts[j]
        if col_ranks >= rpc or col_edges + d > 128:
            col += 1
            col_edges = 0
            col_ranks = 0
        node_col[j] = col
        node_rank[j] = col_ranks
        col_ranks += 1
        col_edges += d
    return node_col, node_rank


def _prepare(x, z, batch, edge_index, solvent_class,
             c_emb, h_emb,
             cW1, cb1, cW2, cb2, cW3, cb3,
             hW1, hb1, hW2, hb2, hW3, hb3):
    x = np.asarray(x, np.float32)
    z = np.asarray(z).reshape(-1).astype(np.int64)
    batch = np.asarray(batch).reshape(-1).astype(np.int64)
    edge_index = np.asarray(edge_index).astype(np.int64)
    solvent_class = np.asarray(solvent_class).reshape(-1).astype(np.int64)
    c_emb = np.asarray(c_emb, np.float32)
    h_emb = np.asarray(h_emb, np.float32)

    src, dst = edge_index[0], edge_index[1]
    valid = (z[src] == 5) & (z[dst] == 0)
    vs, vd = src[valid], dst[valid]
    sol_node = solvent_class[batch]

    order = np.argsort(vs, kind="stable")
    vs, vd = vs[order], vd[order]

    # replicated weights in device layout (fp16 wire format), packed into
    # three const blobs: wmain [128, 2*1540], ue9 [9, 512], bias [128, 16].
    # The x-part of W1 (rows EMB:FH) feeds the main contraction; the emb part
    # is pre-folded with the embedding tables into ue9 = emb @ W1[:EMB].
    wparts = []
    ueparts = []
    bias = np.zeros((128, 16), np.float32)
    for si, (emb, W1, b1, W2, b2, W3, b3) in enumerate((
        (c_emb, cW1, cb1, cW2, cb2, cW3, cb3),
        (h_emb, hW1, hb1, hW2, hb2, hW3, hb3),
    )):
        W1 = np.asarray(W1, np.float32)
        W2 = np.asarray(W2, np.float32)
        W3 = np.asarray(W3, np.float32)
        wparts.append((
            np.concatenate([W1[EMB : EMB + 128, :], W1[EMB + 128 : FH, :]], axis=1),
            np.concatenate([W2[0:128, :], W2[128:256, :]], axis=1),
            np.ascontiguousarray(W3[:, 0].reshape(4, 128).T),
        ))
        ueparts.append(emb @ W1[0:EMB, :])
        bo = 8 * si
        bias[:, bo : bo + 2] = np.asarray(b1, np.float32).reshape(2, 128).T
        bias[:, bo + 2 : bo + 6] = np.asarray(b2, np.float32).reshape(4, 128).T
        bias[0, bo + 6] = np.asarray(b3, np.float32).reshape(-1)[0]
    wdata = {
        "wmain": np.concatenate(
            [wparts[0][0], wparts[1][0], wparts[0][1], wparts[1][1],
             wparts[0][2], wparts[1][2]], axis=1).astype(np.float16),
        "ue9": np.concatenate(ueparts, axis=1).astype(np.float16),
        "bias": bias,
    }

    core_of = vs // CH
    per_core = []
    for c in range(NCORES):
        m = core_of == c
        cs, cd = vs[m], vd[m]
        nodes, counts = np.unique(cs, return_counts=True)
        per_core.append((cs, cd, nodes, counts))

    # smallest geometry that fits every core
    for rpc, ncol in GEOMS:
        packs = [_pack(counts, rpc) for _, _, _, counts in per_core]
        if all(p[0].max(initial=0) < ncol for p in packs):
            break
    else:
        raise ValueError("packing overflow: no geometry fits")
    SLOTS, GRP, NGRP = _geom_params(rpc, ncol)

    in_maps = []
    metas = []
    for c in range(NCORES):
        cs, cd, nodes, counts = per_core[c]
        node_col, node_rank = packs[c]
        ne = len(cs)

        ecol = np.repeat(node_col, counts)
        erank = np.repeat(node_rank, counts)
        einv = np.repeat(1.0 / counts, counts).astype(np.float32)
        ep = np.zeros(ne, np.int64)
        for cc in np.unique(ecol):
            idx = np.nonzero(ecol == cc)[0]
            ep[idx] = np.arange(len(idx))

        vlvw = np.zeros((128, 3 * ncol), np.float32)
        vlvw[ep, ecol] = erank
        vlvw[ep, ncol + ecol] = einv
        vlvw[ep, 2 * ncol + ecol] = sol_node[cd]

        xg3 = np.zeros((128, ncol, HID), np.float16)
        xg3[ep, ecol, :] = x[cd]

        slot = node_col.astype(np.int64) * rpc + node_rank
        xcT = np.zeros((256, SLOTS), np.float16)
        xcT[:, slot] = x[nodes].T
        oh9c = np.zeros((NSOLV, SLOTS), np.float16)
        oh9c[sol_node[nodes], slot] = 1.0

        in_map = dict(
            xg=xg3.reshape(128, ncol * HID),
            xc01=np.concatenate([xcT[0:128], xcT[128:256]], axis=1),
            oh9c=oh9c,
            vlvw=vlvw,
        )
        in_maps.append(in_map)
        metas.append((nodes, slot))
    return in_maps, metas, wdata, (rpc, ncol)


def kernel(**inputs):
    in_maps, metas, wdata, (rpc, ncol) = _prepare(**inputs)
    nc = _get_nc(rpc, ncol, wdata)
    res = bass_utils.run_bass_kernel_spmd(nc, in_maps, core_ids=list(range(NCORES)))
    n = inputs["x"].shape[0]
    out_full = np.zeros((n, 2), np.float32)
    for c in range(NCORES):
        o2 = res.results[c]["out"]  # [2, SLOTS] rows: 0=c, 1=h
        nodes, slot = metas[c]
        out_full[nodes, 0] = o2[0, slot]
        out_full[nodes, 1] = o2[1, slot]
    return out_full


# revision 26
# speedup vs baseline: 1.1992x; 1.1992x over previous
"""Trainium2 Bass kernel for nn_NodeEncodeInterface (GNN message passing).

Strategy (per sharding hint: shard nodes/edges with graph-partitioned edge
cuts, replicate small embeddings + MLP weights):
 - Host: partitions valid carbon->hydrogen edges by owner core (src chunk),
   packs them into static 128-edge columns (<=RPC carbon ranks per column),
   and ships ONLY the x rows each core actually touches, already laid out in
   the packed edge/carbon slot order (fp16 wire format).  The solvent
   embedding is pre-concatenated into each 288-dim feature row, and 1/deg is
   folded into the edge weight, so the device needs no gather, no transpose,
   and no divide.  MLP weights ride inside the NEFF as Const tensors
   (loaded at model-load time, not per-execute).
 - Device (8 NeuronCores, SPMD): computes the segment-mean via
   selection-matrix matmuls in PSUM (fp16 operands, fp32 accumulate), then
   runs both Projection MLPs in transposed orientation, emitting compact
   per-carbon outputs.
 - Host: scatters compact outputs into the full [N, 2] result.
"""

import hashlib

import numpy as np

import concourse.bass as bass
import concourse.mybir as mybir
import concourse.tile as tile_mod
from concourse.tile import TileContext
from concourse.vector_clock import ScopedClock
from concourse import bass_utils

f32 = mybir.dt.float32
f16 = mybir.dt.float16
ALU = mybir.AluOpType

N = 300000
HID = 256
EMB = 32
FH = EMB + HID            # 288 feature dim (emb ++ x)
C2 = FH - 256             # 32: last lhsT chunk of the 288-dim contraction
NCORES = 8
CH = N // NCORES          # 37500 nodes per core

# geometry ladder: smallest (ranks-per-column, n-columns) that fits the
# per-core packing is chosen at runtime (deterministic inputs -> first entry)
GEOMS = ((112, 15), (128, 16), (128, 24), (128, 48))


def _geom_params(rpc, ncol):
    slots = rpc * ncol
    ngrp = -(-slots // 512)
    while slots % ngrp:
        ngrp += 1
    return slots, slots // ngrp, ngrp


# ---------------------------------------------------------------------------
# walrus workaround: this build rejects >1 semaphore wait on several lowered
# instruction encodings; split extra waits onto same-engine NoOps.
# ---------------------------------------------------------------------------
def _patched_drain_and_barrier(self, tick_clock, wait_clock):
    nc = self.nc
    drain_inst = nc.sync.drain()
    wait_clock.add_sem_waits(
        drain_inst.ins, ScopedClock({None: tick_clock.global_clock})
    )
    si = drain_inst.ins.sync_info
    waits = list(si.on_wait)
    if len(waits) > 1:
        si.on_wait = waits[:1]
        for w in waits[1:]:
            extra = nc.sync.drain()
            extra.ins.sync_info = mybir.SyncInfo(on_wait=[w], on_update=[])
    nc.all_engine_barrier()
    popped = nc._tile_sem_poison_stack.pop()
    assert popped is self._sem_poison
    nc.clear_and_free_semaphores(list(self.sems.allocated().values()))
    nc.all_engine_barrier()


tile_mod.TileContext._drain_and_barrier = _patched_drain_and_barrier


def _split_waits(nc, maxw=1):
    fn = nc.m.functions[0]
    for bb in fn.blocks:
        out = []
        changed = False
        for inst in bb.instructions:
            si = inst.sync_info
            waits = list(si.on_wait) if si is not None else []
            if len(waits) > maxw:
                changed = True
                for i in range(0, len(waits) - maxw, maxw):
                    nop = mybir.InstNoOp(
                        name=nc.get_next_instruction_name(),
                        text_hint="waitsplit",
                        bass_nofuse=True,
                    )
                    nop.engine = inst.engine
                    nop.sync_info = mybir.SyncInfo(
                        on_wait=waits[i : i + maxw], on_update=[]
                    )
                    out.append(nop)
                si.on_wait = waits[len(waits) - maxw :]
            out.append(inst)
        if changed:
            bb.instructions[:] = out
    return nc


# ---------------------------------------------------------------------------
# device kernel
# ---------------------------------------------------------------------------
NSOLV = 9


def _build(rpc, ncol, wdata):
    SLOTS, GRP, NGRP = _geom_params(rpc, ncol)
    nc = bass.Bass("TRN2")
    # per-core packed node features (host prepared, fp16); features are the
    # 256-dim x rows only -- the 32-dim solvent embedding is applied via
    # one-hot matmuls against host-precomputed (emb @ W1[:32]) const tables
    xg = nc.dram_tensor("xg", [128, ncol * HID], f16, kind="ExternalInput")
    xc01 = nc.dram_tensor("xc01", [128, 2 * SLOTS], f16, kind="ExternalInput")
    oh9c = nc.dram_tensor("oh9c", [NSOLV, SLOTS], f16, kind="ExternalInput")
    vlvw = nc.dram_tensor("vlvw", [128, 3 * ncol], f32, kind="ExternalInput")
    # replicated MLP weights: Const tensors embedded in the NEFF
    wmain_d = nc.inline_tensor(wdata["wmain"], name="k_wmain")
    ue9_d = nc.inline_tensor(wdata["ue9"], name="k_ue9")
    bias_d = nc.inline_tensor(wdata["bias"], name="k_bias")
    out = nc.dram_tensor("out", [2, SLOTS], f32, kind="ExternalOutput")

    with TileContext(nc) as tc:
        with (
            tc.tile_pool(name="const", bufs=1) as cst,
            tc.tile_pool(name="wts", bufs=1) as wts,
            tc.tile_pool(name="edge", bufs=1) as edg,
            tc.tile_pool(name="work", bufs=3) as wrk,
            tc.tile_pool(name="hsum", bufs=1) as hsp,
            tc.tile_pool(name="mlp", bufs=2) as mlp,
            tc.tile_pool(name="psE", bufs=2, space="PSUM") as psE,
            tc.tile_pool(name="psS", bufs=1, space="PSUM") as psS,
            tc.tile_pool(name="psL", bufs=2, space="PSUM") as psL,
            tc.tile_pool(name="outp", bufs=1) as outp,
        ):
            iota = cst.tile([128, rpc], mybir.dt.int32)
            nc.gpsimd.iota(iota[:], pattern=[[1, rpc]], base=0, channel_multiplier=0)
            iotaf = cst.tile([128, rpc], f32)
            nc.vector.tensor_copy(iotaf[:], iota[:])
            iota9 = cst.tile([128, NSOLV], mybir.dt.int32)
            nc.gpsimd.iota(iota9[:], pattern=[[1, NSOLV]], base=0, channel_multiplier=0)
            iota9f = cst.tile([128, NSOLV], f32)
            nc.vector.tensor_copy(iota9f[:], iota9[:])

            # ---- inputs to SBUF, ordered so the c-side MLP (which needs
            # only w1/xc01) can start while xg/w2 still stream in ----
            wm = wts.tile([128, 2 * 1540], f16)
            nc.sync.dma_start(out=wm[:, 0:1024], in_=wmain_d[:, 0:1024])
            ue9 = wts.tile([NSOLV, 512], f16)
            nc.sync.dma_start(out=ue9[:], in_=ue9_d[:])
            bias = wts.tile([128, 16], f32)
            nc.sync.dma_start(out=bias[:], in_=bias_d[:])
            vlvwT = edg.tile([128, 3 * ncol], f32)
            nc.sync.dma_start(out=vlvwT[:], in_=vlvw[:])
            xc01T = edg.tile([128, 2 * SLOTS], f16)
            nc.sync.dma_start(out=xc01T[:], in_=xc01[:])
            oh9cT = edg.tile([NSOLV, SLOTS], f16)
            nc.sync.dma_start(out=oh9cT[:], in_=oh9c[:])
            nc.sync.dma_start(out=wm[:, 1024:3080], in_=wmain_d[:, 1024:3080])
            xgT = edg.tile([128, ncol * HID], f16)
            nc.sync.dma_start(out=xgT[:], in_=xg[:])

            hs01T = hsp.tile([128, 2 * SLOTS], f16)
            hs9T = hsp.tile([NSOLV, SLOTS], f16)
            o2c = outp.tile([1, SLOTS], f32)
            o2h = outp.tile([1, SLOTS], f32)

            h1t = {}
            h2t = {}
            p3t = {}

            def emit_l1(s, wo, r01, r9, bo, g):
                gs = slice(g * GRP, (g + 1) * GRP)
                gs1 = slice(SLOTS + g * GRP, SLOTS + (g + 1) * GRP)
                h1t[s, g] = mlp.tile([128, 2 * GRP], f16, tag="h1" + s, name="h1t" + s)
                b1o = 512 * wo
                for fb in range(2):
                    ph = psL.tile([128, GRP], f32, tag="pl1")
                    nc.tensor.matmul(ph[:], lhsT=wm[:, b1o + fb * 128 : b1o + fb * 128 + 128], rhs=r01[:, gs], start=True, stop=False)
                    nc.tensor.matmul(ph[:], lhsT=wm[:, b1o + 256 + fb * 128 : b1o + 256 + fb * 128 + 128], rhs=r01[:, gs1], start=False, stop=False)
                    nc.tensor.matmul(ph[:], lhsT=ue9[:, 256 * wo + fb * 128 : 256 * wo + (fb + 1) * 128], rhs=r9[:, gs], start=False, stop=True)
                    nc.vector.tensor_scalar(
                        out=h1t[s, g][:, fb * GRP : (fb + 1) * GRP], in0=ph[:],
                        scalar1=bias[:, bo + fb : bo + fb + 1], scalar2=None, op0=ALU.add,
                    )

            def emit_l2l3(s, wo, o2, bo, g):
                gs = slice(g * GRP, (g + 1) * GRP)
                h2t[s, g] = mlp.tile([128, 4 * GRP], f16, tag="h2" + s, name="h2t" + s)
                p3t[s, g] = psS.tile([1, GRP], f32, tag="p3" + s, name="p3t" + s)
                b2o = 1024 + 1024 * wo
                for fb in range(4):
                    p2m = psL.tile([128, GRP], f32, tag="pl2")
                    nc.tensor.matmul(p2m[:], lhsT=wm[:, b2o + fb * 128 : b2o + (fb + 1) * 128], rhs=h1t[s, g][:, 0:GRP], start=True, stop=False)
                    nc.tensor.matmul(p2m[:], lhsT=wm[:, b2o + 512 + fb * 128 : b2o + 512 + (fb + 1) * 128], rhs=h1t[s, g][:, GRP : 2 * GRP], start=False, stop=True)
                    nc.scalar.activation(
                        h2t[s, g][:, fb * GRP : (fb + 1) * GRP], p2m[:],
                        mybir.ActivationFunctionType.Relu, bias=bias[:, bo + 2 + fb : bo + 3 + fb],
                    )
                for fb in range(4):
                    nc.tensor.matmul(
                        p3t[s, g][:], lhsT=wm[:, 3072 + 4 * wo + fb : 3072 + 4 * wo + fb + 1],
                        rhs=h2t[s, g][:, fb * GRP : (fb + 1) * GRP],
                        start=(fb == 0), stop=(fb == 3),
                    )
                nc.vector.tensor_scalar(
                    out=o2[:, gs], in0=p3t[s, g][:],
                    scalar1=bias[0:1, bo + 6 : bo + 7], scalar2=None, op0=ALU.add,
                )

            def emit_side(s, wo, r01, r9, o2, bo):
                for g in range(NGRP):
                    emit_l1(s, wo, r01, r9, bo, g)
                    if g >= 1:
                        emit_l2l3(s, wo, o2, bo, g - 1)
                emit_l2l3(s, wo, o2, bo, NGRP - 1)

            # ---- c-side MLP first: overlaps the xg/w2 input stream ----
            emit_side("c", 0, xc01T, oh9cT, o2c, 0)
            nc.sync.dma_start(out=out[0:1, :], in_=o2c[:])

            # ---- segment mean via selection matmuls ----
            for i in range(ncol):
                Seq = wrk.tile([128, rpc], f32, tag="Seq")
                nc.vector.tensor_tensor(
                    out=Seq[:],
                    in0=vlvwT[:, i : i + 1].to_broadcast([128, rpc]),
                    in1=iotaf[:],
                    op=ALU.is_equal,
                )
                S16 = wrk.tile([128, rpc], f16, tag="S16")
                nc.vector.tensor_scalar(
                    out=S16[:], in0=Seq[:], scalar1=vlvwT[:, ncol + i : ncol + i + 1],
                    scalar2=None, op0=ALU.mult,
                )
                H9 = wrk.tile([128, NSOLV], f16, tag="H9")
                nc.vector.tensor_tensor(
                    out=H9[:],
                    in0=vlvwT[:, 2 * ncol + i : 2 * ncol + i + 1].to_broadcast([128, NSOLV]),
                    in1=iota9f[:],
                    op=ALU.is_equal,
                )
                sl = slice(i * rpc, (i + 1) * rpc)
                pE = psE.tile([128, 3 * rpc], f32, tag="pE")
                base = i * HID
                nc.tensor.matmul(pE[:, 0:rpc], lhsT=xgT[:, base : base + 128], rhs=S16[:], start=True, stop=True)
                nc.tensor.matmul(pE[:, rpc : 2 * rpc], lhsT=xgT[:, base + 128 : base + 256], rhs=S16[:], start=True, stop=True)
                nc.tensor.matmul(pE[0:NSOLV, 2 * rpc : 3 * rpc], lhsT=H9[:], rhs=S16[:], start=True, stop=True)
                nc.vector.tensor_copy(
                    hs01T[:].rearrange("p (c s) -> p c s", c=2)[:, :, sl],
                    pE[:, 0 : 2 * rpc].rearrange("p (c r) -> p c r", c=2),
                )
                nc.vector.tensor_copy(hs9T[:, sl], pE[0:NSOLV, 2 * rpc : 3 * rpc])

            # ---- h-side MLP ----
            emit_side("h", 1, hs01T, hs9T, o2h, 8)
            nc.sync.dma_start(out=out[1:2, :], in_=o2h[:])
    _split_waits(nc)
    return nc


_NC_CACHE = {}


def _get_nc(rpc, ncol, wdata):
    h = hashlib.sha1()
    for k in sorted(wdata):
        h.update(k.encode())
        h.update(wdata[k].tobytes())
    key = (rpc, ncol, h.hexdigest())
    if key not in _NC_CACHE:
        _NC_CACHE[key] = _build(rpc, ncol, wdata)
    return _NC_CACHE[key]


# ---------------------------------------------------------------------------
# host side
# ---------------------------------------------------------------------------
def _pack(counts, rpc):
    """Greedy packing of nodes (with edge multiplicities `counts`) into
    columns of <=128 edges and <=rpc ranks.  Returns (node_col, node_rank)."""
    k = len(counts)
    node_col = np.zeros(k, np.int32)
    node_rank = np.zeros(k, np.int32)
    col = 0
    col_edges = 0
    col_ranks = 0
    for j in range(k):
        d = counts[j]
        if col_ranks >= rpc or col_edges + d > 128:
            col += 1
            col_edges = 0
            col_ranks = 0
        node_col[j] = col
        node_rank[j] = col_ranks
        col_ranks += 1
        col_edges += d
    return node_col, node_rank


def _prepare(x, z, batch, edge_index, solvent_class,
             c_emb, h_emb,
             cW1, cb1, cW2, cb2, cW3, cb3,
             hW1, hb1, hW2, hb2, hW3, hb3):
    x = np.asarray(x, np.float32)
    z = np.asarray(z).reshape(-1).astype(np.int64)
    batch = np.asarray(batch).reshape(-1).astype(np.int64)
    edge_index = np.asarray(edge_index).astype(np.int64)
    solvent_class = np.asarray(solvent_class).reshape(-1).astype(np.int64)
    c_emb = np.asarray(c_emb, np.float32)
    h_emb = np.asarray(h_emb, np.float32)

    src, dst = edge_index[0], edge_index[1]
    valid = (z[src] == 5) & (z[dst] == 0)
    vs, vd = src[valid], dst[valid]
    sol_node = solvent_class[batch]

    order = np.argsort(vs, kind="stable")
    vs, vd = vs[order], vd[order]

    # replicated weights in device layout (fp16 wire format), packed into
    # three const blobs: wmain [128, 2*1540], ue9 [9, 512], bias [128, 16].
    # The x-part of W1 (rows EMB:FH) feeds the main contraction; the emb part
    # is pre-folded with the embedding tables into ue9 = emb @ W1[:EMB].
    wparts = []
    ueparts = []
    bias = np.zeros((128, 16), np.float32)
    for si, (emb, W1, b1, W2, b2, W3, b3) in enumerate((
        (c_emb, cW1, cb1, cW2, cb2, cW3, cb3),
        (h_emb, hW1, hb1, hW2, hb2, hW3, hb3),
    )):
        W1 = np.asarray(W1, np.float32)
        W2 = np.asarray(W2, np.float32)
        W3 = np.asarray(W3, np.float32)
        wparts.append((
            np.concatenate([W1[EMB : EMB + 128, :], W1[EMB + 128 : FH, :]], axis=1),
            np.concatenate([W2[0:128, :], W2[128:256, :]], axis=1),
            np.ascontiguousarray(W3[:, 0].reshape(4, 128).T),
        ))
        ueparts.append(emb @ W1[0:EMB, :])
        bo = 8 * si
        bias[:, bo : bo + 2] = np.asarray(b1, np.float32).reshape(2, 128).T
        bias[:, bo + 2 : bo + 6] = np.asarray(b2, np.float32).reshape(4, 128).T
        bias[0, bo + 6] = np.asarray(b3, np.float32).reshape(-1)[0]
    wdata = {
        "wmain": np.concatenate(
            [wparts[0][0], wparts[1][0], wparts[0][1], wparts[1][1],
             wparts[0][2], wparts[1][2]], axis=1).astype(np.float16),
        "ue9": np.concatenate(ueparts, axis=1).astype(np.float16),
        "bias": bias,
    }

    core_of = vs // CH
    per_core = []
    for c in range(NCORES):
        m = core_of == c
        cs, cd = vs[m], vd[m]
        nodes, counts = np.unique(cs, return_counts=True)
        per_core.append((cs, cd, nodes, counts))

    # smallest geometry that fits every core
    for rpc, ncol in GEOMS:
        packs = [_pack(counts, rpc) for _, _, _, counts in per_core]
        if all(p[0].max(initial=0) < ncol for p in packs):
            break
    else:
        raise ValueError("packing overflow: no geometry fits")
    SLOTS, GRP, NGRP = _geom_params(rpc, ncol)

    in_maps = []
    metas = []
    for c in range(NCORES):
        cs, cd, nodes, counts = per_core[c]
        node_col, node_rank = packs[c]
        ne = len(cs)

        ecol = np.repeat(node_col, counts)
        erank = np.repeat(node_rank, counts)
        einv = np.repeat(1.0 / counts, counts).astype(np.float32)
        ep = np.zeros(ne, np.int64)
        for cc in np.unique(ecol):
            idx = np.nonzero(ecol == cc)[0]
            ep[idx] = np.arange(len(idx))

        vlvw = np.zeros((128, 3 * ncol), np.float32)
        vlvw[ep, ecol] = erank
        vlvw[ep, ncol + ecol] = einv
        vlvw[ep, 2 * ncol + ecol] = sol_node[cd]

        xg3 = np.zeros((128, ncol, HID), np.float16)
        xg3[ep, ecol, :] = x[cd]

        slot = node_col.astype(np.int64) * rpc + node_rank
        xcT = np.zeros((256, SLOTS), np.float16)
        xcT[:, slot] = x[nodes].T
        oh9c = np.zeros((NSOLV, SLOTS), np.float16)
        oh9c[sol_node[nodes], slot] = 1.0

        in_map = dict(
            xg=xg3.reshape(128, ncol * HID),
            xc01=np.concatenate([xcT[0:128], xcT[128:256]], axis=1),
            oh9c=oh9c,
            vlvw=vlvw,
        )
        in_maps.append(in_map)
        metas.append((nodes, slot))
    return in_maps, metas, wdata, (rpc, ncol)


def kernel(**inputs):
    in_maps, metas, wdata, (rpc, ncol) = _prepare(**inputs)
    nc = _get_nc(rpc, ncol, wdata)
    res = bass_utils.run_bass_kernel_spmd(nc, in_maps, core_ids=list(range(NCORES)))
    n = inputs["x"].shape[0]
    out_full = np.zeros((n, 2), np.float32)
    for c in range(NCORES):
        o2 = res.results[c]["out"]  # [2, SLOTS] rows: 0=c, 1=h
        nodes, slot = metas[c]
        out_full[nodes, 0] = o2[0, slot]
        out_full[nodes, 1] = o2[1, slot]
    return out_full


# revision 27
# speedup vs baseline: 1.2016x; 1.0019x over previous
"""Trainium2 Bass kernel for nn_NodeEncodeInterface (GNN message passing).

Strategy (per sharding hint: shard nodes/edges with graph-partitioned edge
cuts, replicate small embeddings + MLP weights):
 - Host: partitions valid carbon->hydrogen edges by owner core (src chunk),
   packs them into static 128-edge columns (<=RPC carbon ranks per column),
   and ships ONLY the x rows each core actually touches, already laid out in
   the packed edge/carbon slot order (fp16 wire format).  The solvent
   embedding is pre-concatenated into each 288-dim feature row, and 1/deg is
   folded into the edge weight, so the device needs no gather, no transpose,
   and no divide.  MLP weights ride inside the NEFF as Const tensors
   (loaded at model-load time, not per-execute).
 - Device (8 NeuronCores, SPMD): computes the segment-mean via
   selection-matrix matmuls in PSUM (fp16 operands, fp32 accumulate), then
   runs both Projection MLPs in transposed orientation, emitting compact
   per-carbon outputs.
 - Host: scatters compact outputs into the full [N, 2] result.
"""

import hashlib

import numpy as np

import concourse.bass as bass
import concourse.mybir as mybir
import concourse.tile as tile_mod
from concourse.tile import TileContext
from concourse.vector_clock import ScopedClock
from concourse import bass_utils

f32 = mybir.dt.float32
f16 = mybir.dt.float16
ALU = mybir.AluOpType

N = 300000
HID = 256
EMB = 32
FH = EMB + HID            # 288 feature dim (emb ++ x)
C2 = FH - 256             # 32: last lhsT chunk of the 288-dim contraction
NCORES = 8
CH = N // NCORES          # 37500 nodes per core

# geometry ladder: smallest (ranks-per-column, n-columns) that fits the
# per-core packing is chosen at runtime (deterministic inputs -> first entry)
GEOMS = ((112, 15), (128, 16), (128, 24), (128, 48))


def _geom_params(rpc, ncol):
    slots = rpc * ncol
    ngrp = -(-slots // 512)
    while slots % ngrp:
        ngrp += 1
    return slots, slots // ngrp, ngrp


# ---------------------------------------------------------------------------
# walrus workaround: this build rejects >1 semaphore wait on several lowered
# instruction encodings; split extra waits onto same-engine NoOps.
# ---------------------------------------------------------------------------
def _patched_drain_and_barrier(self, tick_clock, wait_clock):
    nc = self.nc
    drain_inst = nc.sync.drain()
    wait_clock.add_sem_waits(
        drain_inst.ins, ScopedClock({None: tick_clock.global_clock})
    )
    si = drain_inst.ins.sync_info
    waits = list(si.on_wait)
    if len(waits) > 1:
        si.on_wait = waits[:1]
        for w in waits[1:]:
            extra = nc.sync.drain()
            extra.ins.sync_info = mybir.SyncInfo(on_wait=[w], on_update=[])
    nc.all_engine_barrier()
    popped = nc._tile_sem_poison_stack.pop()
    assert popped is self._sem_poison
    nc.clear_and_free_semaphores(list(self.sems.allocated().values()))
    nc.all_engine_barrier()


tile_mod.TileContext._drain_and_barrier = _patched_drain_and_barrier


def _split_waits(nc, maxw=1):
    fn = nc.m.functions[0]
    for bb in fn.blocks:
        out = []
        changed = False
        for inst in bb.instructions:
            si = inst.sync_info
            waits = list(si.on_wait) if si is not None else []
            if len(waits) > maxw:
                changed = True
                for i in range(0, len(waits) - maxw, maxw):
                    nop = mybir.InstNoOp(
                        name=nc.get_next_instruction_name(),
                        text_hint="waitsplit",
                        bass_nofuse=True,
                    )
                    nop.engine = inst.engine
                    nop.sync_info = mybir.SyncInfo(
                        on_wait=waits[i : i + maxw], on_update=[]
                    )
                    out.append(nop)
                si.on_wait = waits[len(waits) - maxw :]
            out.append(inst)
        if changed:
            bb.instructions[:] = out
    return nc


# ---------------------------------------------------------------------------
# device kernel
# ---------------------------------------------------------------------------
NSOLV = 9


def _build(rpc, ncol, wdata):
    SLOTS, GRP, NGRP = _geom_params(rpc, ncol)
    nc = bass.Bass("TRN2")
    # per-core packed node features (host prepared, fp16); features are the
    # 256-dim x rows only -- the 32-dim solvent embedding is applied via
    # one-hot matmuls against host-precomputed (emb @ W1[:32]) const tables
    xg = nc.dram_tensor("xg", [128, ncol * HID], f16, kind="ExternalInput")
    xc01 = nc.dram_tensor("xc01", [128, 2 * SLOTS], f16, kind="ExternalInput")
    oh9c = nc.dram_tensor("oh9c", [NSOLV, SLOTS], f16, kind="ExternalInput")
    vlvw = nc.dram_tensor("vlvw", [128, 3 * ncol], f32, kind="ExternalInput")
    # replicated MLP weights: Const tensors embedded in the NEFF
    wmain_d = nc.inline_tensor(wdata["wmain"], name="k_wmain")
    ue9_d = nc.inline_tensor(wdata["ue9"], name="k_ue9")
    bias_d = nc.inline_tensor(wdata["bias"], name="k_bias")
    out = nc.dram_tensor("out", [2, SLOTS], f32, kind="ExternalOutput")

    with TileContext(nc) as tc:
        with (
            tc.tile_pool(name="const", bufs=1) as cst,
            tc.tile_pool(name="wts", bufs=1) as wts,
            tc.tile_pool(name="edge", bufs=1) as edg,
            tc.tile_pool(name="work", bufs=3) as wrk,
            tc.tile_pool(name="hsum", bufs=1) as hsp,
            tc.tile_pool(name="mlp", bufs=2) as mlp,
            tc.tile_pool(name="psE", bufs=2, space="PSUM") as psE,
            tc.tile_pool(name="psS", bufs=1, space="PSUM") as psS,
            tc.tile_pool(name="psL", bufs=2, space="PSUM") as psL,
            tc.tile_pool(name="outp", bufs=1) as outp,
        ):
            iota = cst.tile([128, rpc], mybir.dt.int32)
            nc.gpsimd.iota(iota[:], pattern=[[1, rpc]], base=0, channel_multiplier=0)
            iotaf = cst.tile([128, rpc], f32)
            nc.vector.tensor_copy(iotaf[:], iota[:])
            iota9 = cst.tile([128, NSOLV], mybir.dt.int32)
            nc.gpsimd.iota(iota9[:], pattern=[[1, NSOLV]], base=0, channel_multiplier=0)
            iota9f = cst.tile([128, NSOLV], f32)
            nc.vector.tensor_copy(iota9f[:], iota9[:])

            # ---- inputs to SBUF, ordered so the c-side MLP (which needs
            # only w1/xc01) can start while xg/w2 still stream in ----
            wm = wts.tile([128, 2 * 1540], f16)
            nc.sync.dma_start(out=wm[:, 0:1024], in_=wmain_d[:, 0:1024])
            ue9 = wts.tile([NSOLV, 512], f16)
            nc.sync.dma_start(out=ue9[:], in_=ue9_d[:])
            bias = wts.tile([128, 16], f32)
            nc.sync.dma_start(out=bias[:], in_=bias_d[:])
            vlvwT = edg.tile([128, 3 * ncol], f32)
            nc.sync.dma_start(out=vlvwT[:], in_=vlvw[:])
            oh9cT = edg.tile([NSOLV, SLOTS], f16)
            nc.sync.dma_start(out=oh9cT[:], in_=oh9c[:])
            # xc01 is group-major on the wire: [g0 chunk0|chunk1, g1 ...] so
            # each half is one contiguous DMA and group 0 unblocks early
            xc01T = edg.tile([128, 2 * SLOTS], f16)
            nc.sync.dma_start(out=xc01T[:, 0 : 4 * GRP], in_=xc01[:, 0 : 4 * GRP])
            nc.sync.dma_start(out=xc01T[:, 4 * GRP : 8 * GRP], in_=xc01[:, 4 * GRP : 8 * GRP])
            nc.sync.dma_start(out=wm[:, 1024:3080], in_=wmain_d[:, 1024:3080])
            xgT = edg.tile([128, ncol * HID], f16)
            nc.sync.dma_start(out=xgT[:], in_=xg[:])

            hs01T = hsp.tile([128, 2 * SLOTS], f16)
            hs9T = hsp.tile([NSOLV, SLOTS], f16)
            o2c = outp.tile([1, SLOTS], f32)
            o2h = outp.tile([1, SLOTS], f32)

            h1t = {}
            h2t = {}
            p3t = {}

            def emit_l1(s, wo, r01, r9, bo, g):
                gs = slice(g * GRP, (g + 1) * GRP)
                if s == "c":  # group-major wire layout
                    gs0 = slice(2 * g * GRP, (2 * g + 1) * GRP)
                    gs1 = slice((2 * g + 1) * GRP, (2 * g + 2) * GRP)
                else:  # chunk-major (written by the seg phase)
                    gs0 = gs
                    gs1 = slice(SLOTS + g * GRP, SLOTS + (g + 1) * GRP)
                h1t[s, g] = mlp.tile([128, 2 * GRP], f16, tag="h1" + s, name="h1t" + s)
                b1o = 512 * wo
                for fb in range(2):
                    ph = psL.tile([128, GRP], f32, tag="pl1")
                    nc.tensor.matmul(ph[:], lhsT=wm[:, b1o + fb * 128 : b1o + fb * 128 + 128], rhs=r01[:, gs0], start=True, stop=False)
                    nc.tensor.matmul(ph[:], lhsT=wm[:, b1o + 256 + fb * 128 : b1o + 256 + fb * 128 + 128], rhs=r01[:, gs1], start=False, stop=False)
                    nc.tensor.matmul(ph[:], lhsT=ue9[:, 256 * wo + fb * 128 : 256 * wo + (fb + 1) * 128], rhs=r9[:, gs], start=False, stop=True)
                    nc.vector.tensor_scalar(
                        out=h1t[s, g][:, fb * GRP : (fb + 1) * GRP], in0=ph[:],
                        scalar1=bias[:, bo + fb : bo + fb + 1], scalar2=None, op0=ALU.add,
                    )

            def emit_l2l3(s, wo, o2, bo, g):
                gs = slice(g * GRP, (g + 1) * GRP)
                h2t[s, g] = mlp.tile([128, 4 * GRP], f16, tag="h2" + s, name="h2t" + s)
                p3t[s, g] = psS.tile([1, GRP], f32, tag="p3" + s, name="p3t" + s)
                b2o = 1024 + 1024 * wo
                for fb in range(4):
                    p2m = psL.tile([128, GRP], f32, tag="pl2")
                    nc.tensor.matmul(p2m[:], lhsT=wm[:, b2o + fb * 128 : b2o + (fb + 1) * 128], rhs=h1t[s, g][:, 0:GRP], start=True, stop=False)
                    nc.tensor.matmul(p2m[:], lhsT=wm[:, b2o + 512 + fb * 128 : b2o + 512 + (fb + 1) * 128], rhs=h1t[s, g][:, GRP : 2 * GRP], start=False, stop=True)
                    nc.scalar.activation(
                        h2t[s, g][:, fb * GRP : (fb + 1) * GRP], p2m[:],
                        mybir.ActivationFunctionType.Relu, bias=bias[:, bo + 2 + fb : bo + 3 + fb],
                    )
                for fb in range(4):
                    nc.tensor.matmul(
                        p3t[s, g][:], lhsT=wm[:, 3072 + 4 * wo + fb : 3072 + 4 * wo + fb + 1],
                        rhs=h2t[s, g][:, fb * GRP : (fb + 1) * GRP],
                        start=(fb == 0), stop=(fb == 3),
                    )
                nc.vector.tensor_scalar(
                    out=o2[:, gs], in0=p3t[s, g][:],
                    scalar1=bias[0:1, bo + 6 : bo + 7], scalar2=None, op0=ALU.add,
                )

            def emit_side(s, wo, r01, r9, o2, bo):
                for g in range(NGRP):
                    emit_l1(s, wo, r01, r9, bo, g)
                    if g >= 1:
                        emit_l2l3(s, wo, o2, bo, g - 1)
                emit_l2l3(s, wo, o2, bo, NGRP - 1)

            # ---- c-side MLP first: overlaps the xg/w2 input stream ----
            emit_side("c", 0, xc01T, oh9cT, o2c, 0)
            nc.sync.dma_start(out=out[0:1, :], in_=o2c[:])

            # ---- segment mean via selection matmuls ----
            for i in range(ncol):
                Seq = wrk.tile([128, rpc], f32, tag="Seq")
                nc.vector.tensor_tensor(
                    out=Seq[:],
                    in0=vlvwT[:, i : i + 1].to_broadcast([128, rpc]),
                    in1=iotaf[:],
                    op=ALU.is_equal,
                )
                S16 = wrk.tile([128, rpc], f16, tag="S16")
                nc.vector.tensor_scalar(
                    out=S16[:], in0=Seq[:], scalar1=vlvwT[:, ncol + i : ncol + i + 1],
                    scalar2=None, op0=ALU.mult,
                )
                H9 = wrk.tile([128, NSOLV], f16, tag="H9")
                nc.vector.tensor_tensor(
                    out=H9[:],
                    in0=vlvwT[:, 2 * ncol + i : 2 * ncol + i + 1].to_broadcast([128, NSOLV]),
                    in1=iota9f[:],
                    op=ALU.is_equal,
                )
                sl = slice(i * rpc, (i + 1) * rpc)
                pE = psE.tile([128, 3 * rpc], f32, tag="pE")
                base = i * HID
                nc.tensor.matmul(pE[:, 0:rpc], lhsT=xgT[:, base : base + 128], rhs=S16[:], start=True, stop=True)
                nc.tensor.matmul(pE[:, rpc : 2 * rpc], lhsT=xgT[:, base + 128 : base + 256], rhs=S16[:], start=True, stop=True)
                nc.tensor.matmul(pE[0:NSOLV, 2 * rpc : 3 * rpc], lhsT=H9[:], rhs=S16[:], start=True, stop=True)
                nc.vector.tensor_copy(
                    hs01T[:].rearrange("p (c s) -> p c s", c=2)[:, :, sl],
                    pE[:, 0 : 2 * rpc].rearrange("p (c r) -> p c r", c=2),
                )
                nc.vector.tensor_copy(hs9T[:, sl], pE[0:NSOLV, 2 * rpc : 3 * rpc])

            # ---- h-side MLP ----
            emit_side("h", 1, hs01T, hs9T, o2h, 8)
            nc.sync.dma_start(out=out[1:2, :], in_=o2h[:])
    _split_waits(nc)
    return nc


_NC_CACHE = {}


def _get_nc(rpc, ncol, wdata):
    h = hashlib.sha1()
    for k in sorted(wdata):
        h.update(k.encode())
        h.update(wdata[k].tobytes())
    key = (rpc, ncol, h.hexdigest())
    if key not in _NC_CACHE:
        _NC_CACHE[key] = _build(rpc, ncol, wdata)
    return _NC_CACHE[key]


# ---------------------------------------------------------------------------
# host side
# ---------------------------------------------------------------------------
def _pack(counts, rpc):
    """Greedy packing of nodes (with edge multiplicities `counts`) into
    columns of <=128 edges and <=rpc ranks.  Returns (node_col, node_rank)."""
    k = len(counts)
    node_col = np.zeros(k, np.int32)
    node_rank = np.zeros(k, np.int32)
    col = 0
    col_edges = 0
    col_ranks = 0
    for j in range(k):
        d = counts[j]
        if col_ranks >= rpc or col_edges + d > 128:
            col += 1
            col_edges = 0
            col_ranks = 0
        node_col[j] = col
        node_rank[j] = col_ranks
        col_ranks += 1
        col_edges += d
    return node_col, node_rank


def _prepare(x, z, batch, edge_index, solvent_class,
             c_emb, h_emb,
             cW1, cb1, cW2, cb2, cW3, cb3,
             hW1, hb1, hW2, hb2, hW3, hb3):
    x = np.asarray(x, np.float32)
    z = np.asarray(z).reshape(-1).astype(np.int64)
    batch = np.asarray(batch).reshape(-1).astype(np.int64)
    edge_index = np.asarray(edge_index).astype(np.int64)
    solvent_class = np.asarray(solvent_class).reshape(-1).astype(np.int64)
    c_emb = np.asarray(c_emb, np.float32)
    h_emb = np.asarray(h_emb, np.float32)

    src, dst = edge_index[0], edge_index[1]
    valid = (z[src] == 5) & (z[dst] == 0)
    vs, vd = src[valid], dst[valid]
    sol_node = solvent_class[batch]

    order = np.argsort(vs, kind="stable")
    vs, vd = vs[order], vd[order]

    # replicated weights in device layout (fp16 wire format), packed into
    # three const blobs: wmain [128, 2*1540], ue9 [9, 512], bias [128, 16].
    # The x-part of W1 (rows EMB:FH) feeds the main contraction; the emb part
    # is pre-folded with the embedding tables into ue9 = emb @ W1[:EMB].
    wparts = []
    ueparts = []
    bias = np.zeros((128, 16), np.float32)
    for si, (emb, W1, b1, W2, b2, W3, b3) in enumerate((
        (c_emb, cW1, cb1, cW2, cb2, cW3, cb3),
        (h_emb, hW1, hb1, hW2, hb2, hW3, hb3),
    )):
        W1 = np.asarray(W1, np.float32)
        W2 = np.asarray(W2, np.float32)
        W3 = np.asarray(W3, np.float32)
        wparts.append((
            np.concatenate([W1[EMB : EMB + 128, :], W1[EMB + 128 : FH, :]], axis=1),
            np.concatenate([W2[0:128, :], W2[128:256, :]], axis=1),
            np.ascontiguousarray(W3[:, 0].reshape(4, 128).T),
        ))
        ueparts.append(emb @ W1[0:EMB, :])
        bo = 8 * si
        bias[:, bo : bo + 2] = np.asarray(b1, np.float32).reshape(2, 128).T
        bias[:, bo + 2 : bo + 6] = np.asarray(b2, np.float32).reshape(4, 128).T
        bias[0, bo + 6] = np.asarray(b3, np.float32).reshape(-1)[0]
    wdata = {
        "wmain": np.concatenate(
            [wparts[0][0], wparts[1][0], wparts[0][1], wparts[1][1],
             wparts[0][2], wparts[1][2]], axis=1).astype(np.float16),
        "ue9": np.concatenate(ueparts, axis=1).astype(np.float16),
        "bias": bias,
    }

    core_of = vs // CH
    per_core = []
    for c in range(NCORES):
        m = core_of == c
        cs, cd = vs[m], vd[m]
        nodes, counts = np.unique(cs, return_counts=True)
        per_core.append((cs, cd, nodes, counts))

    # smallest geometry that fits every core
    for rpc, ncol in GEOMS:
        packs = [_pack(counts, rpc) for _, _, _, counts in per_core]
        if all(p[0].max(initial=0) < ncol for p in packs):
            break
    else:
        raise ValueError("packing overflow: no geometry fits")
    SLOTS, GRP, NGRP = _geom_params(rpc, ncol)

    in_maps = []
    metas = []
    for c in range(NCORES):
        cs, cd, nodes, counts = per_core[c]
        node_col, node_rank = packs[c]
        ne = len(cs)

        ecol = np.repeat(node_col, counts)
        erank = np.repeat(node_rank, counts)
        einv = np.repeat(1.0 / counts, counts).astype(np.float32)
        ep = np.zeros(ne, np.int64)
        for cc in np.unique(ecol):
            idx = np.nonzero(ecol == cc)[0]
            ep[idx] = np.arange(len(idx))

        vlvw = np.zeros((128, 3 * ncol), np.float32)
        vlvw[ep, ecol] = erank
        vlvw[ep, ncol + ecol] = einv
        vlvw[ep, 2 * ncol + ecol] = sol_node[cd]

        xg3 = np.zeros((128, ncol, HID), np.float16)
        xg3[ep, ecol, :] = x[cd]

        slot = node_col.astype(np.int64) * rpc + node_rank
        xcT = np.zeros((256, SLOTS), np.float16)
        xcT[:, slot] = x[nodes].T
        oh9c = np.zeros((NSOLV, SLOTS), np.float16)
        oh9c[sol_node[nodes], slot] = 1.0

        in_map = dict(
            xg=xg3.reshape(128, ncol * HID),
            xc01=np.concatenate(
                [xcT[c * 128 : (c + 1) * 128, g * GRP : (g + 1) * GRP]
                 for g in range(NGRP) for c in range(2)], axis=1),
            oh9c=oh9c,
            vlvw=vlvw,
        )
        in_maps.append(in_map)
        metas.append((nodes, slot))
    return in_maps, metas, wdata, (rpc, ncol)


def kernel(**inputs):
    in_maps, metas, wdata, (rpc, ncol) = _prepare(**inputs)
    nc = _get_nc(rpc, ncol, wdata)
    res = bass_utils.run_bass_kernel_spmd(nc, in_maps, core_ids=list(range(NCORES)))
    n = inputs["x"].shape[0]
    out_full = np.zeros((n, 2), np.float32)
    for c in range(NCORES):
        o2 = res.results[c]["out"]  # [2, SLOTS] rows: 0=c, 1=h
        nodes, slot = metas[c]
        out_full[nodes, 0] = o2[0, slot]
        out_full[nodes, 1] = o2[1, slot]
    return out_full
